# revision 1
# baseline (speedup 1.0000x reference)
"""Multi-head attention (B=2, S=2048, D=1024, H=16) on 8 NeuronCores.

Sharding: core c -> (batch b = c//4, head-group hg = c%4 of 4 heads).
Each core computes QKV projection for its 4 heads (bf16 matmuls, f32 PSUM),
transposed-score flash attention (S^T = K^T-tile.T-stationary @ Q^T streams,
softmax denominator via an appended ones-column on V), and the output
projection restricted to its heads' rows of out_w.  The host sums the 4
per-head-group partial outputs per batch and adds out_b (exact, linear).

Device layouts (per core):
  xt  [D(+1), S]  bf16   x[b]^T (+ ones row when qkv_b != 0)
  w   [D(+1), 768] bf16  qkv_w columns for this core's heads (q|k|v) (+ bias row)
  wo  [256, D] bf16      out_w rows for this core's heads
  out [S, D] f32         partial output (sum over the 4 head-groups = x-slice
                         contribution; host adds groups + out_b)
"""

import os
import sys

sys.path.insert(0, "/opt/trn_rl_repo")

import numpy as np
import ml_dtypes

import concourse.bass as bass  # noqa: F401  (AP helpers)
import concourse.mybir as mybir
import concourse.tile as tile
from concourse import bacc
from concourse.bass_utils import run_bass_kernel_spmd
from concourse.masks import make_upper_triangular

B, S, D, H, DH = 2, 2048, 1024, 16, 64
NCORES = 8
HPC = 4            # heads per core
EQ = HPC * DH      # 256: q (or k, or v) columns per core
E = 3 * EQ         # 768: total projected columns per core
BF16 = mybir.dt.bfloat16
F32 = mybir.dt.float32
NP_BF16 = ml_dtypes.bfloat16
EXPFN = mybir.ActivationFunctionType.Exp

_prog_cache: dict = {}
last_results = None  # BassKernelResults of the most recent run (for test.py)


def _emit(tc, xt_h, w_h, wo_h, out_h, causal, dd):
    nc = tc.nc
    nd = (dd + 127) // 128          # number of contraction sub-tiles
    dsubs = [(i * 128, min(128, dd - i * 128)) for i in range(nd)]

    with (
        tc.tile_pool(name="persist", bufs=1) as pp,
        tc.tile_pool(name="pt", bufs=3) as pt_pool,
        tc.tile_pool(name="norm", bufs=2) as norm_pool,
        tc.tile_pool(name="outsb", bufs=3) as out_pool,
    ):
        # ---- persistent SBUF tensors ----
        xt_sb = pp.tile([128, nd, S], BF16, tag="xt", name="xt_sb")
        w_sb = pp.tile([128, nd, E], BF16, tag="w", name="w_sb")
        wo_sb = pp.tile([128, 2, D], BF16, tag="wo", name="wo_sb")
        qkT_sb = pp.tile([128, 4, S], BF16, tag="qkT", name="qkT_sb")
        # V' per (k-tile j, head h): [128, 65], col 64 = ones (softmax denom)
        vp_sb = pp.tile([128, 16, HPC, 65], BF16, tag="vp", name="vp_sb")
        ctx_all = pp.tile([128, 2, S], BF16, tag="ctx", name="ctx_all")

        for d, (o, ln) in enumerate(dsubs):
            nc.sync.dma_start(out=xt_sb[0:ln, d, :], in_=xt_h[o : o + ln, :])
            nc.sync.dma_start(out=w_sb[0:ln, d, :], in_=w_h[o : o + ln, :])
        for et in range(2):
            nc.sync.dma_start(
                out=wo_sb[:, et, :], in_=wo_h[128 * et : 128 * (et + 1), :]
            )

        if causal:
            tri_f = pp.tile([128, 128], F32, tag="trif", name="tri_f")
            make_upper_triangular(nc, tri_f, val=1.0, diag=True)
            tri_bf = pp.tile([128, 128], BF16, tag="trib", name="tri_bf")
            nc.vector.tensor_copy(tri_bf, tri_f)

        nc.vector.memset(vp_sb[:, :, :, 64:65], 1.0)

        # ---- phase 1: QKV^T projection ----
        # qkT_sb[:, 0:2, :] = Q^T e-tiles, [:, 2:4, :] = K^T e-tiles.
        with tc.tile_pool(name="pj", bufs=2, space="PSUM") as pjp:
            for et in range(4):
                for half in range(2):
                    ps = pjp.tile([128, 1024], F32, tag="qk", name="pj_ps")
                    for d in range(nd):
                        ln = dsubs[d][1]
                        for c in range(2):
                            nc.tensor.matmul(
                                ps[:, 512 * c : 512 * (c + 1)],
                                lhsT=w_sb[0:ln, d, 128 * et : 128 * (et + 1)],
                                rhs=xt_sb[
                                    0:ln,
                                    d,
                                    1024 * half + 512 * c : 1024 * half + 512 * (c + 1),
                                ],
                                start=(d == 0),
                                stop=(d == nd - 1),
                            )
                    nc.vector.tensor_copy(
                        qkT_sb[:, et, 1024 * half : 1024 * (half + 1)], ps
                    )
            # V directly in [s, e_v] layout: lhsT = x^T tile (stationary)
            for i in range(16):
                vps = pjp.tile([128, 256], F32, tag="v", name="v_ps")
                for d in range(nd):
                    ln = dsubs[d][1]
                    nc.tensor.matmul(
                        vps,
                        lhsT=xt_sb[0:ln, d, 128 * i : 128 * (i + 1)],
                        rhs=w_sb[0:ln, d, 2 * EQ : 3 * EQ],
                        start=(d == 0),
                        stop=(d == nd - 1),
                    )
                nc.vector.tensor_copy(
                    vp_sb[:, i, :, 0:64],
                    vps.rearrange("p (h e) -> p h e", h=HPC),
                )

        # ---- phase 2: attention per head ----
        with (
            tc.tile_pool(name="ctxp", bufs=1, space="PSUM") as ctxp,
            tc.tile_pool(name="stp", bufs=2, space="PSUM") as stp,
        ):
            for h in range(HPC):
                po = (h % 2) * 64  # partition offset inside the e-tile pair
                qsl = qkT_sb[po : po + 64, h // 2, :]
                ksl = qkT_sb[po : po + 64, 2 + h // 2, :]
                ctx_ps = ctxp.tile([65, S], F32, tag="ctx", name="ctx_ps")
                for j in range(16):
                    q0 = 128 * j if causal else 0
                    c0 = q0 // 512
                    zlen = q0 - 512 * c0
                    pt = pt_pool.tile([128, S], BF16, tag="pt", name="pt_t")
                    if zlen:
                        nc.vector.memset(pt[:, 0:zlen], 0.0)
                    pos = q0
                    while pos < S:
                        plen = min(1024, S - pos)
                        st = stp.tile([128, 1024], F32, tag="st", name="st_ps")
                        off = 0
                        while off < plen:
                            clen = min(512, plen - off)
                            nc.tensor.matmul(
                                st[:, off : off + clen],
                                lhsT=ksl[:, 128 * j : 128 * (j + 1)],
                                rhs=qsl[:, pos + off : pos + off + clen],
                                start=True,
                                stop=True,
                            )
                            off += clen
                        nc.scalar.activation(
                            pt[:, pos - 512 * c0 : pos - 512 * c0 + plen],
                            st[:, 0:plen],
                            EXPFN,
                            scale=0.125,
                        )
                        pos += plen
                    if causal:
                        nc.vector.tensor_mul(
                            pt[:, zlen : zlen + 128], pt[:, zlen : zlen + 128], tri_bf
                        )
                    for c in range(c0, 4):
                        jl = min(15, 4 * c + 3) if causal else 15
                        nc.tensor.matmul(
                            ctx_ps[:, 512 * c : 512 * (c + 1)],
                            lhsT=vp_sb[:, j, h, :],
                            rhs=pt[:, 512 * (c - c0) : 512 * (c - c0 + 1)],
                            start=(j == 0),
                            stop=(j == jl),
                        )
                # normalize: ctx[q, :] /= den[q]; den sits on psum partition 64
                den = norm_pool.tile([1, S], F32, tag="den", name="den_sb")
                nc.vector.tensor_copy(den, ctx_ps[64:65, :])
                denb = norm_pool.tile([64, S], F32, tag="denb", name="denb_sb")
                nc.gpsimd.partition_broadcast(denb, den)
                rec = norm_pool.tile([64, S], F32, tag="rec", name="rec_sb")
                nc.vector.reciprocal(rec, denb)
                nc.vector.tensor_mul(
                    ctx_all[po : po + 64, h // 2, :], ctx_ps[0:64, :], rec
                )

        # ---- phase 3: output projection ----
        with tc.tile_pool(name="op", bufs=3, space="PSUM") as op:
            for i in range(16):
                ops = op.tile([128, 1024], F32, tag="o", name="o_ps")
                for et in range(2):
                    for c in range(2):
                        nc.tensor.matmul(
                            ops[:, 512 * c : 512 * (c + 1)],
                            lhsT=ctx_all[:, et, 128 * i : 128 * (i + 1)],
                            rhs=wo_sb[:, et, 512 * c : 512 * (c + 1)],
                            start=(et == 0),
                            stop=(et == 1),
                        )
                osb = out_pool.tile([128, 1024], F32, tag="osb", name="o_sb")
                nc.vector.tensor_copy(osb, ops)
                nc.sync.dma_start(out=out_h[128 * i : 128 * (i + 1), :], in_=osb)


def _get_prog(causal: bool, dd: int):
    key = (causal, dd)
    if key not in _prog_cache:
        nc = bacc.Bacc("TRN2", target_bir_lowering=False, debug=False)
        xt_h = nc.dram_tensor("xt", [dd, S], BF16, kind="ExternalInput")
        w_h = nc.dram_tensor("w", [dd, E], BF16, kind="ExternalInput")
        wo_h = nc.dram_tensor("wo", [EQ, D], BF16, kind="ExternalInput")
        out_h = nc.dram_tensor("out", [S, D], F32, kind="ExternalOutput")
        with tile.TileContext(nc) as tc:
            _emit(tc, xt_h, w_h, wo_h, out_h, causal, dd)
        nc.compile()
        _prog_cache[key] = nc
    return _prog_cache[key]


def _numpy_fallback(x, mask, qkv_w, qkv_b, out_w, out_b):
    qkv = x.reshape(B * S, D) @ qkv_w + qkv_b
    qkv = qkv.reshape(B, S, 3, H, DH)
    q, k, v = qkv[:, :, 0], qkv[:, :, 1], qkv[:, :, 2]
    sc = np.einsum("bqhd,bkhd->bhqk", q, k) / np.sqrt(np.float32(DH))
    sc = np.where(mask, sc, np.float32(-1e9))
    sc = sc - sc.max(-1, keepdims=True)
    a = np.exp(sc)
    a = a / a.sum(-1, keepdims=True)
    ctx = np.einsum("bhqk,bkhd->bqhd", a, v).reshape(B, S, D)
    return (ctx.reshape(B * S, D) @ out_w + out_b).reshape(B, S, D).astype(np.float32)


def kernel(x, mask, qkv_w, qkv_b, out_w, out_b):
    global last_results
    x = np.asarray(x, dtype=np.float32)
    mask = np.asarray(mask).astype(bool)
    qkv_w = np.asarray(qkv_w, dtype=np.float32)
    qkv_b = np.asarray(qkv_b, dtype=np.float32)
    out_w = np.asarray(out_w, dtype=np.float32)
    out_b = np.asarray(out_b, dtype=np.float32)

    m2 = mask.reshape(S, S)
    if m2.all():
        causal = False
    elif np.array_equal(m2, np.tril(np.ones((S, S), dtype=bool))):
        causal = True
    else:
        return _numpy_fallback(x, mask, qkv_w, qkv_b, out_w, out_b)

    has_b = bool(np.any(qkv_b))
    dd = D + 1 if has_b else D
    nc = _get_prog(causal, dd)

    in_maps = []
    for c in range(NCORES):
        b, hg = divmod(c, 4)
        hs = hg * HPC
        cols = slice(hs * DH, (hs + HPC) * DH)
        wc = np.concatenate(
            [qkv_w[:, cols], qkv_w[:, D:][:, cols], qkv_w[:, 2 * D :][:, cols]], axis=1
        )
        xtc = x[b].T
        if has_b:
            bc = np.concatenate(
                [qkv_b[cols], qkv_b[D:][cols], qkv_b[2 * D :][cols]]
            )
            wc = np.concatenate([wc, bc[None, :]], axis=0)
            xtc = np.concatenate([xtc, np.ones((1, S), np.float32)], axis=0)
        in_maps.append(
            {
                "xt": np.ascontiguousarray(xtc).astype(NP_BF16),
                "w": np.ascontiguousarray(wc).astype(NP_BF16),
                "wo": np.ascontiguousarray(out_w[cols, :]).astype(NP_BF16),
            }
        )

    trace = os.environ.get("KERNEL_TRACE", "0") == "1"
    last_results = run_bass_kernel_spmd(
        nc, in_maps, core_ids=list(range(NCORES)), trace=trace
    )
    out = np.zeros((B, S, D), dtype=np.float32)
    for c in range(NCORES):
        out[c // 4] += last_results.results[c]["out"]
    out += out_b[None, None, :]
    return out


# revision 6
# speedup vs baseline: 1.0500x; 1.0500x over previous
"""Multi-head attention (B=2, S=2048, D=1024, H=16) on 8 NeuronCores.

Sharding: core c -> (batch b = c//4, head-group hg = c%4 of 4 heads).
Each core computes QKV projection for its 4 heads (bf16 matmuls, f32 PSUM),
transposed-score flash attention (S^T = K^T-tile.T-stationary @ Q^T streams,
softmax denominator via an appended ones-column on V), and the output
projection restricted to its heads' rows of out_w.  The host sums the 4
per-head-group partial outputs per batch and adds out_b (exact, linear).

Device layouts (per core):
  xt  [D(+1), S]  bf16   x[b]^T (+ ones row when qkv_b != 0)
  w   [D(+1), 768] bf16  qkv_w columns for this core's heads (q|k|v) (+ bias row)
  wo  [256, D] bf16      out_w rows for this core's heads
  out [S, D] f32         partial output (sum over the 4 head-groups = x-slice
                         contribution; host adds groups + out_b)
"""

import os
import sys

sys.path.insert(0, "/opt/trn_rl_repo")

import numpy as np
import ml_dtypes

import concourse.bass as bass  # noqa: F401  (AP helpers)
import concourse.mybir as mybir
import concourse.tile as tile
from concourse import bacc
from concourse.bass_utils import run_bass_kernel_spmd
from concourse.masks import make_upper_triangular

B, S, D, H, DH = 2, 2048, 1024, 16, 64
NCORES = 8
HPC = 4            # heads per core
EQ = HPC * DH      # 256: q (or k, or v) columns per core
E = 3 * EQ         # 768: total projected columns per core
BF16 = mybir.dt.bfloat16
F32 = mybir.dt.float32
NP_BF16 = ml_dtypes.bfloat16
EXPFN = mybir.ActivationFunctionType.Exp

_prog_cache: dict = {}
last_results = None  # BassKernelResults of the most recent run (for test.py)


def _emit(tc, xt_h, w_h, wo_h, out_h, causal, dd):
    nc = tc.nc
    nd = (dd + 127) // 128          # number of contraction sub-tiles
    dsubs = [(i * 128, min(128, dd - i * 128)) for i in range(nd)]

    with (
        tc.tile_pool(name="persist", bufs=1) as pp,
        tc.tile_pool(name="pt", bufs=4) as pt_pool,
        tc.tile_pool(name="norm", bufs=2) as norm_pool,
        tc.tile_pool(name="outsb", bufs=3) as out_pool,
    ):
        # ---- persistent SBUF tensors ----
        xt_sb = pp.tile([128, nd, S], BF16, tag="xt", name="xt_sb")
        w_sb = pp.tile([128, nd, E], BF16, tag="w", name="w_sb")
        wo_sb = pp.tile([128, 2, D], BF16, tag="wo", name="wo_sb")
        qkT_sb = pp.tile([128, 4, S], BF16, tag="qkT", name="qkT_sb")
        # V' per (k-tile j, head h): [128, 65], col 64 = ones (softmax denom)
        vp_sb = pp.tile([128, 16, HPC, 65], BF16, tag="vp", name="vp_sb")
        ctx_all = pp.tile([128, 2, S], BF16, tag="ctx", name="ctx_all")

        for d, (o, ln) in enumerate(dsubs):
            nc.sync.dma_start(out=xt_sb[0:ln, d, :], in_=xt_h[o : o + ln, :])
            nc.sync.dma_start(out=w_sb[0:ln, d, :], in_=w_h[o : o + ln, :])
        for et in range(2):
            nc.sync.dma_start(
                out=wo_sb[:, et, :], in_=wo_h[128 * et : 128 * (et + 1), :]
            )

        if causal:
            tri_f = pp.tile([128, 128], F32, tag="trif", name="tri_f")
            make_upper_triangular(nc, tri_f, val=1.0, diag=True)
            tri_bf = pp.tile([128, 128], BF16, tag="trib", name="tri_bf")
            nc.vector.tensor_copy(tri_bf, tri_f)

        nc.vector.memset(vp_sb[:, :, :, 64:65], 1.0)

        # ---- phase 1: QKV^T projection ----
        # qkT_sb[:, 0:2, :] = Q^T e-tiles, [:, 2:4, :] = K^T e-tiles.
        with tc.tile_pool(name="pj", bufs=2, space="PSUM") as pjp:
            for et in range(4):
                for half in range(2):
                    ps = pjp.tile([128, 1024], F32, tag="qk", name="pj_ps")
                    for d in range(nd):
                        ln = dsubs[d][1]
                        for c in range(2):
                            nc.tensor.matmul(
                                ps[:, 512 * c : 512 * (c + 1)],
                                lhsT=w_sb[0:ln, d, 128 * et : 128 * (et + 1)],
                                rhs=xt_sb[
                                    0:ln,
                                    d,
                                    1024 * half + 512 * c : 1024 * half + 512 * (c + 1),
                                ],
                                start=(d == 0),
                                stop=(d == nd - 1),
                            )
                    nc.scalar.copy(
                        qkT_sb[:, et, 1024 * half : 1024 * (half + 1)], ps
                    )
            # V directly in [s, e_v] layout: lhsT = x^T tile (stationary)
            for i in range(16):
                vps = pjp.tile([128, 256], F32, tag="v", name="v_ps")
                for d in range(nd):
                    ln = dsubs[d][1]
                    nc.tensor.matmul(
                        vps,
                        lhsT=xt_sb[0:ln, d, 128 * i : 128 * (i + 1)],
                        rhs=w_sb[0:ln, d, 2 * EQ : 3 * EQ],
                        start=(d == 0),
                        stop=(d == nd - 1),
                    )
                nc.scalar.copy(
                    vp_sb[:, i, :, 0:64],
                    vps.rearrange("p (h e) -> p h e", h=HPC),
                )

        # ---- phase 2: attention per head ----
        with (
            tc.tile_pool(name="ctxp", bufs=1, space="PSUM") as ctxp,
            tc.tile_pool(name="stp", bufs=2, space="PSUM") as stp,
        ):
            for h in range(HPC):
                po = (h % 2) * 64  # partition offset inside the e-tile pair
                qsl = qkT_sb[po : po + 64, h // 2, :]
                ksl = qkT_sb[po : po + 64, 2 + h // 2, :]
                ctx_ps = ctxp.tile([65, S], F32, tag="ctx", name="ctx_ps")
                for j in range(16):
                    q0 = 128 * j if causal else 0
                    c0 = q0 // 512
                    zlen = q0 - 512 * c0
                    pt = pt_pool.tile([128, S], BF16, tag="pt", name="pt_t")
                    # pt covers q in [q0, S): column q -> pt[:, q - q0]
                    pos = q0
                    while pos < S:
                        plen = min(1024, S - pos)
                        st = stp.tile([128, 1024], F32, tag="st", name="st_ps")
                        off = 0
                        while off < plen:
                            clen = min(512, plen - off)
                            nc.tensor.matmul(
                                st[:, off : off + clen],
                                lhsT=ksl[:, 128 * j : 128 * (j + 1)],
                                rhs=qsl[:, pos + off : pos + off + clen],
                                start=True,
                                stop=True,
                            )
                            off += clen
                        nc.scalar.activation(
                            pt[:, pos - q0 : pos - q0 + plen],
                            st[:, 0:plen],
                            EXPFN,
                            scale=0.125,
                        )
                        pos += plen
                    if causal:
                        nc.vector.tensor_mul(
                            pt[:, 0:128], pt[:, 0:128], tri_bf
                        )
                    for c in range(c0, 4):
                        jl = min(15, 4 * c + 3) if causal else 15
                        if c == c0:
                            dst = ctx_ps[:, 512 * c0 + zlen : 512 * (c0 + 1)]
                            src = pt[:, 0 : 512 - zlen]
                        else:
                            dst = ctx_ps[:, 512 * c : 512 * (c + 1)]
                            src = pt[:, 512 * (c - c0) - zlen : 512 * (c - c0 + 1) - zlen]
                        nc.tensor.matmul(
                            dst,
                            lhsT=vp_sb[:, j, h, :],
                            rhs=src,
                            start=(j == 0),
                            stop=(j == jl),
                        )
                # Evacuate unnormalized ctx^T (+ denominator row 64) to SBUF
                # immediately so the PSUM banks free up for the next head;
                # the normalization chain below runs off the critical path.
                ctxu = norm_pool.tile([65, S], F32, tag="ctxu", name="ctxu_sb")
                nc.vector.tensor_copy(ctxu, ctx_ps)
                rec = norm_pool.tile([1, S], F32, tag="rec", name="rec_sb")
                nc.vector.reciprocal(rec, ctxu[64:65, :])
                recb = norm_pool.tile([64, S], F32, tag="recb", name="recb_sb")
                nc.gpsimd.partition_broadcast(recb, rec)
                nc.vector.tensor_mul(
                    ctx_all[po : po + 64, h // 2, :], ctxu[0:64, :], recb
                )

        # ---- phase 3: output projection ----
        with tc.tile_pool(name="op", bufs=3, space="PSUM") as op:
            for i in range(16):
                ops = op.tile([128, 1024], F32, tag="o", name="o_ps")
                for et in range(2):
                    for c in range(2):
                        nc.tensor.matmul(
                            ops[:, 512 * c : 512 * (c + 1)],
                            lhsT=ctx_all[:, et, 128 * i : 128 * (i + 1)],
                            rhs=wo_sb[:, et, 512 * c : 512 * (c + 1)],
                            start=(et == 0),
                            stop=(et == 1),
                        )
                osb = out_pool.tile([128, 1024], F32, tag="osb", name="o_sb")
                nc.vector.tensor_copy(osb, ops)
                nc.sync.dma_start(out=out_h[128 * i : 128 * (i + 1), :], in_=osb)


def _get_prog(causal: bool, dd: int):
    key = (causal, dd)
    if key not in _prog_cache:
        nc = bacc.Bacc("TRN2", target_bir_lowering=False, debug=False)
        xt_h = nc.dram_tensor("xt", [dd, S], BF16, kind="ExternalInput")
        w_h = nc.dram_tensor("w", [dd, E], BF16, kind="ExternalInput")
        wo_h = nc.dram_tensor("wo", [EQ, D], BF16, kind="ExternalInput")
        out_h = nc.dram_tensor("out", [S, D], F32, kind="ExternalOutput")
        with tile.TileContext(nc) as tc:
            _emit(tc, xt_h, w_h, wo_h, out_h, causal, dd)
        nc.compile()
        _prog_cache[key] = nc
    return _prog_cache[key]


def _numpy_fallback(x, mask, qkv_w, qkv_b, out_w, out_b):
    qkv = x.reshape(B * S, D) @ qkv_w + qkv_b
    qkv = qkv.reshape(B, S, 3, H, DH)
    q, k, v = qkv[:, :, 0], qkv[:, :, 1], qkv[:, :, 2]
    sc = np.einsum("bqhd,bkhd->bhqk", q, k) / np.sqrt(np.float32(DH))
    sc = np.where(mask, sc, np.float32(-1e9))
    sc = sc - sc.max(-1, keepdims=True)
    a = np.exp(sc)
    a = a / a.sum(-1, keepdims=True)
    ctx = np.einsum("bhqk,bkhd->bqhd", a, v).reshape(B, S, D)
    return (ctx.reshape(B * S, D) @ out_w + out_b).reshape(B, S, D).astype(np.float32)


def kernel(x, mask, qkv_w, qkv_b, out_w, out_b):
    global last_results
    x = np.asarray(x, dtype=np.float32)
    mask = np.asarray(mask).astype(bool)
    qkv_w = np.asarray(qkv_w, dtype=np.float32)
    qkv_b = np.asarray(qkv_b, dtype=np.float32)
    out_w = np.asarray(out_w, dtype=np.float32)
    out_b = np.asarray(out_b, dtype=np.float32)

    m2 = mask.reshape(S, S)
    if m2.all():
        causal = False
    elif np.array_equal(m2, np.tril(np.ones((S, S), dtype=bool))):
        causal = True
    else:
        return _numpy_fallback(x, mask, qkv_w, qkv_b, out_w, out_b)

    has_b = bool(np.any(qkv_b))
    dd = D + 1 if has_b else D
    nc = _get_prog(causal, dd)

    in_maps = []
    for c in range(NCORES):
        b, hg = divmod(c, 4)
        hs = hg * HPC
        cols = slice(hs * DH, (hs + HPC) * DH)
        wc = np.concatenate(
            [qkv_w[:, cols], qkv_w[:, D:][:, cols], qkv_w[:, 2 * D :][:, cols]], axis=1
        )
        xtc = x[b].T
        if has_b:
            bc = np.concatenate(
                [qkv_b[cols], qkv_b[D:][cols], qkv_b[2 * D :][cols]]
            )
            wc = np.concatenate([wc, bc[None, :]], axis=0)
            xtc = np.concatenate([xtc, np.ones((1, S), np.float32)], axis=0)
        in_maps.append(
            {
                "xt": np.ascontiguousarray(xtc).astype(NP_BF16),
                "w": np.ascontiguousarray(wc).astype(NP_BF16),
                "wo": np.ascontiguousarray(out_w[cols, :]).astype(NP_BF16),
            }
        )

    trace = os.environ.get("KERNEL_TRACE", "0") == "1"
    last_results = run_bass_kernel_spmd(
        nc, in_maps, core_ids=list(range(NCORES)), trace=trace
    )
    out = np.zeros((B, S, D), dtype=np.float32)
    for c in range(NCORES):
        out[c // 4] += last_results.results[c]["out"]
    out += out_b[None, None, :]
    return out
